# revision 17
# baseline (speedup 1.0000x reference)
# kernel_pair.py — pair-merged variant: GPSIMD ops span 2 mega-tiles.
# Race-oracle probing showed GPSIMD per-op overhead ~1.4 us (its streaming
# rate is near line-rate), so its 16 ops are merged into 8 double-size ops
# over image-pairs, and the row split rebalanced: GPSIMD takes out rows
# 0..29 (top), DVE rows 30..61.
import numpy as np

N_CORES = 8
N, C = 16, 256
H = W = 64
OH = OW = 62
P = 128
IMGS_PER_CORE = (N * C) // N_CORES    # 512
NPAIR = 2                             # pairs of 128-image tiles

G_OUT = 29                            # GPSIMD out rows 0..28
G_MID = G_OUT + 2                     # mid rows 0..31  (x rows 0..31)
D_OUT = OH - G_OUT                    # DVE out rows 30..61 (32)
D_MID = H - G_OUT                     # DVE mid rows 30..63 (34)

_nc_cache = {}


def _split_multiwait(nc, max_waits=1):
    import concourse.mybir as mb

    for f in nc.m.functions:
        for b in f.blocks:
            new_list = []
            for inst in b.instructions:
                si = getattr(inst, "sync_info", None)
                if si is not None and len(si.on_wait) > max_waits:
                    waits = list(si.on_wait)
                    extra, keep = waits[:-max_waits], waits[-max_waits:]
                    for k, w in enumerate(extra):
                        es = mb.InstEventSemaphore(
                            name=f"{inst.name}-esw{k}", ins=[], outs=[],
                            engine=inst.engine)
                        es.sync_info = mb.SyncInfo(on_wait=[w], on_update=[])
                        nc.register_instruction(es)
                        new_list.append(es)
                    inst.sync_info = mb.SyncInfo(
                        on_wait=keep, on_update=list(si.on_update))
                new_list.append(inst)
            b.instructions[:] = new_list


def _build_nc():
    import concourse.bass as bass
    import concourse.mybir as mybir
    from concourse.tile import TileContext

    f32 = mybir.dt.float32
    g, gm, d, dm = G_OUT, G_MID, D_OUT, D_MID

    nc = bass.Bass()
    x = nc.declare_dram_parameter("x", [IMGS_PER_CORE, H, W], f32, isOutput=False)
    o = nc.declare_dram_parameter("o", [IMGS_PER_CORE, OH, OW], f32, isOutput=True)

    with TileContext(nc) as tc:
        with (
            tc.tile_pool(name="xp", bufs=NPAIR) as xp,
            tc.tile_pool(name="mgp", bufs=NPAIR) as mgp,
            tc.tile_pool(name="mdp", bufs=NPAIR) as mdp,
            tc.tile_pool(name="op", bufs=NPAIR) as op,
        ):
            for p in range(NPAIR):
                i0 = p * 2 * P        # first image of the pair
                xb = xp.tile([P, 2, H, W], f32)
                # GPSIMD's rows (top) first so the merged op starts early,
                # then DVE's rows per member.
                nc.sync.dma_start(out=xb[:, 0, 0:gm, :], in_=x[i0:i0 + P, 0:gm])
                nc.sync.dma_start(out=xb[:, 1, 0:gm, :], in_=x[i0 + P:i0 + 2 * P, 0:gm])
                nc.sync.dma_start(out=xb[:, 0, gm:H, :], in_=x[i0:i0 + P, gm:H])
                nc.sync.dma_start(out=xb[:, 1, gm:H, :], in_=x[i0 + P:i0 + 2 * P, gm:H])

                ot = op.tile([P, 2, OH, OW], f32)

                # ---- GPSIMD: pair-merged 4-op chain, out rows 0..g-1 ----
                mg = mgp.tile([P, 2, gm, OW], f32)
                nc.gpsimd.tensor_add(
                    out=mg[:], in0=xb[:, :, 0:gm, 0:62], in1=xb[:, :, 0:gm, 1:63])
                nc.gpsimd.tensor_add(
                    out=mg[:], in0=mg[:], in1=xb[:, :, 0:gm, 2:64])
                nc.gpsimd.tensor_add(
                    out=ot[:, :, 0:g, :], in0=mg[:, :, 0:g, :], in1=mg[:, :, 1:g + 1, :])
                nc.gpsimd.tensor_add(
                    out=ot[:, :, 0:g, :], in0=ot[:, :, 0:g, :], in1=mg[:, :, 2:g + 2, :])

                # ---- DVE: per-member chains, out rows g..61; per-member
                # 1/9 on ScalarE right after each chain so stores fire early
                nc.scalar.mul(out=ot[:, :, 0:g, :], in_=ot[:, :, 0:g, :], mul=1.0 / 9.0)
                last = (p == NPAIR - 1)
                for m in range(2):
                    md = mdp.tile([P, dm, OW], f32)
                    nc.vector.tensor_add(
                        out=md[:], in0=xb[:, m, g:H, 0:62], in1=xb[:, m, g:H, 1:63])
                    nc.vector.tensor_add(
                        out=md[:], in0=md[:], in1=xb[:, m, g:H, 2:64])
                    nc.vector.tensor_add(
                        out=ot[:, m, g:OH, :], in0=md[:, 0:d, :], in1=md[:, 1:d + 1, :])
                    if last and m == 1:
                        # final chain: split V2/scale at row RS so the last
                        # store is half-sized (shorter pipeline tail)
                        rs = 46
                        nc.vector.tensor_add(
                            out=ot[:, 1, g:rs, :], in0=ot[:, 1, g:rs, :],
                            in1=md[:, 2:rs - g + 2, :])
                        nc.scalar.mul(out=ot[:, 1, g:rs, :],
                                      in_=ot[:, 1, g:rs, :], mul=1.0 / 9.0)
                        nc.vector.tensor_add(
                            out=ot[:, 1, rs:OH, :], in0=ot[:, 1, rs:OH, :],
                            in1=md[:, rs - g + 2:d + 2, :])
                        nc.scalar.mul(out=ot[:, 1, rs:OH, :],
                                      in_=ot[:, 1, rs:OH, :], mul=1.0 / 9.0)
                    else:
                        nc.vector.tensor_add(
                            out=ot[:, m, g:OH, :], in0=ot[:, m, g:OH, :],
                            in1=md[:, 2:d + 2, :])
                        nc.scalar.mul(out=ot[:, m, g:OH, :],
                                      in_=ot[:, m, g:OH, :], mul=1.0 / 9.0)

                # stores per member (contiguous runs per partition)
                nc.sync.dma_start(out=o[i0:i0 + P], in_=ot[:, 0, :, :])
                if last:
                    # final two stores on different HWDGE rings: they drain
                    # in parallel instead of serializing on one ring
                    rs = 46
                    nc.sync.dma_start(
                        out=o[i0 + P:i0 + 2 * P, 0:rs, :], in_=ot[:, 1, 0:rs, :])
                    nc.scalar.dma_start(
                        out=o[i0 + P:i0 + 2 * P, rs:OH, :], in_=ot[:, 1, rs:OH, :])
                else:
                    nc.sync.dma_start(out=o[i0 + P:i0 + 2 * P], in_=ot[:, 1, :, :])

    _split_multiwait(nc)
    nc.finalize()
    return nc


def _get_nc():
    if "nc" not in _nc_cache:
        _nc_cache["nc"] = _build_nc()
    return _nc_cache["nc"]


def run(x, trace=False, **spmd_kwargs):
    from concourse.bass_utils import run_bass_kernel_spmd

    x = np.ascontiguousarray(np.asarray(x, dtype=np.float32))
    assert x.shape == (N, C, H, W), x.shape
    shards = x.reshape(N_CORES, IMGS_PER_CORE, H, W)
    in_maps = [{"x": shards[c]} for c in range(N_CORES)]
    nc = _get_nc()
    res = run_bass_kernel_spmd(
        nc, in_maps, list(range(N_CORES)), trace=trace, **spmd_kwargs
    )
    out = np.stack([res.results[c]["o"] for c in range(N_CORES)], axis=0)
    return out.reshape(N, C, OH, OW), res


def kernel(x):
    out, _ = run(x, trace=False)
    return out
